# revision 30
# baseline (speedup 1.0000x reference)
"""Trainium2 Bass kernel for 3D windowed (3x3x3) per-channel softmax attention.

Problem (hardcoded): x (1,32,24,48,64) f32; Wq/Wk/Wv (48,32); rel_* (16,...,3).
  q = Wq@x ; kf/vf = Wk/Wv @ pad(x) ; per (c,voxel): softmax over the 27
  window taps of q*(k_win+rel), then weighted sum of v_win.

v8 strategy (v7 "type-pure passes" + pow-path rebalance; ~1.2x over v7):
  - Shard D=24 across 8 cores (3 output d-slices + 1-voxel halo, zero-padded
    on host). Host pre-tiles each core's slab into 8 overlapping H-blocks.
  - rel structure: channels 0:16 vary only with wj, 16:32 only with dj,
    32:48 only with hj. Pass p therefore takes channel type p over ALL 8
    blocks: 8 blocks x 16 chans = exactly 128 rows, and every row's rel
    takes just 3 values (one per axis index).
  - Projections: host stacks 4 blocks' x on the 128 partitions (k=128);
    a block-diagonal lhsT projects 4 blocks x 16 chans in ONE matmul per
    64-row half. The k evacuation is an ACT Identity+bias copy that lands
    k + rel[axis=0] directly (variant V0); V1/V2 = V0 + delta via DVE
    tensor_scalar (4x mode).
  - e-production via exp(q·k) = exp(q)^k: an f32 tile E = exp(q) (one ACT
    exp per pass) lets Pool produce whole e-subgroups with a single
    TensorTensor pow(E_win, k_win) — no DVE logits mult, no ACT exp.
    The remaining subgroups keep the DVE-mult + ACT-exp path so all four
    engines stay saturated.
  - e*v on DVE/Pool per a static balance map; one DVE pair-merge of the
    ev planes per subgroup trims the PE's num-reduction from 3 to 2
    planes. Both softmax reductions ride the otherwise-idle PE as chains
    of identity matmuls accumulating bf16 planes into f32 PSUM banks.
  - Staging of pass p+1 is emitted between pass p's groups; finals
    (reciprocal on DVE, normalize-mult on Pool, DMA out) are deferred
    into the next pass's first group.
"""

import sys

sys.path.insert(0, "/opt/trn_rl_repo")

import numpy as np

import concourse.bass as bass
import concourse.bacc as bacc
import concourse.mybir as mybir
import concourse.tile as tile
from concourse.bass_utils import run_bass_kernel_spmd

# ---- problem constants (hardcoded per contract) ----
B, CIN, D, H, W = 1, 32, 24, 48, 64
COUT, K, C3 = 48, 3, 16
NCORES = 8
DLOC = D // NCORES            # 3 output d-slices per core
DP = DLOC + 2                 # 5 padded d-planes per core
NS = 8                        # H-blocks per core
HB = H // NS                  # 6 output rows per block
HBP = HB + 2                  # 8 padded rows per block
WP = W + 2                    # 66
BLK = DP * HBP * WP           # 2640 padded voxels per block
NJ = K * K * K                # 27
NPASS = 3
FL = (HB - 1) * WP + W        # 394-elem flat (h,w) span per d-plane
NV = DLOC * FL                # 1182 per scratch plane

F32 = mybir.dt.float32
BF16 = mybir.dt.bfloat16

# ---- per-pass, per-subgroup balance maps (subgroup s = 3*g + hj2) ----
# pass 2 stages nothing (no pass 3), so its ACT/PE have slack: shift
# production toward the ACT path and drop the DVE merges there.
_P = frozenset({0, 3, 5, 8})
_E = frozenset({1, 2, 4, 6})
_M = frozenset({0, 1, 2, 3, 4, 5, 7})
PATHP = (_P, _P, _P)                   # e via Pool pow(E, kwin)
EV_POOL = (_E, _E, _E)                 # e*v on Pool (rest on DVE)
MERGE_EV = (_M, _M, _M)                # DVE-merge ev planes 0+1
MERGE_SCR = frozenset()                # DVE-merge scr planes 0+1 (den side)
MERGE_POOL = frozenset()               # subgroups whose ev-merge runs on Pool
VAR_ACT = frozenset()                  # passes whose variant-adds ride ACT
FMUL_OFF = frozenset()                 # chunks evac'd on ACT + mult'd on Pool
NUM_DEFER = 1                          # num-chain deferral depth (subgroups)
EV_BUFS = 3                            # ev tile ring depth
HJ2_ORDER = (0, 1, 2)                  # subgroup processing order per group
USE_DIVIDE = False                     # finals: DVE divide vs recip+mult

_CACHE = {}


def _row_map():
    """m[p][r] = (h_block, chan). Pass p = channel type p:
    row r -> block r//16, channel 16p + r%16."""
    return [[(r // 16, 16 * p + r % 16) for r in range(128)]
            for p in range(NPASS)]


def _axis_of(p, j):
    """Variant index used by tap j=(dj,hj,wj) in pass p (= channel type)."""
    dj, hj, wj = j // 9, (j // 3) % 3, j % 3
    return (wj, dj, hj)[p]


def build_program():
    nc = bacc.Bacc("TRN2", target_bir_lowering=False, debug=False, num_devices=NCORES)

    # x4: partitions 32g:32g+32 = x of block 4u+g  (u = 64-row half)
    x4 = nc.declare_dram_parameter("x4", [128, 2, DP, HBP, WP], BF16, isOutput=False)
    # block-diagonal lhsT: idx = 3*kvq + p, rows 32g+cin -> cols 16g+i
    wproj = nc.declare_dram_parameter("wproj", [128, 9, 64], BF16, isOutput=False)
    # relv[p, r, a] = rel value of row r's channel at axis-index a (f32)
    relv = nc.declare_dram_parameter("relv", [NPASS, 128, K], F32, isOutput=False)
    ident = nc.declare_dram_parameter("ident", [128, 128], BF16, isOutput=False)
    y = nc.declare_dram_parameter("y", [NPASS, 128, DLOC, HB, W], F32, isOutput=True)

    with tile.TileContext(nc) as tc:
        with (
            tc.tile_pool(name="consts", bufs=1) as consts,
            tc.tile_pool(name="rows", bufs=1) as rows_pool,
            tc.tile_pool(name="proj", bufs=2, space="PSUM") as proj_ps,
            tc.tile_pool(name="accs", bufs=1, space="PSUM") as accs,
            tc.tile_pool(name="attn", bufs=1) as attn,
            tc.tile_pool(name="outs", bufs=1) as outs,
        ):
            # ---- constants (x4 first on the SP queue; small consts ride
            # the Pool DMA queue so they don't delay the x4 load) ----
            x4_sb = consts.tile([128, 2, DP, HBP, WP], BF16, name="x4_sb")
            for d0, d1 in ((0, 2), (2, 4), (4, 5)):
                for u in range(2):
                    nc.sync.dma_start(out=x4_sb[:, u, d0:d1], in_=x4[:, u, d0:d1])
            wp_sb = consts.tile([128, 9, 64], BF16, name="wp_sb")
            nc.gpsimd.dma_start(out=wp_sb, in_=wproj[:])
            relv_sb = consts.tile([128, NPASS, K], F32, name="relv_sb")
            for p in range(NPASS):
                nc.gpsimd.dma_start(out=relv_sb[:, p], in_=relv[p])
            # f32 per-pass deltas rel[a]-rel[0] for DVE tensor_scalar adds
            reld_sb = consts.tile([128, NPASS, 2], F32, name="reld_sb")
            id_sb = consts.tile([128, 128], BF16, name="id_sb")
            nc.gpsimd.dma_start(out=id_sb, in_=ident[:])

            for p in range(NPASS):
                for a in (1, 2):
                    nc.gpsimd.tensor_tensor(
                        out=reld_sb[:, p, a - 1:a], in0=relv_sb[:, p, a:a + 1],
                        in1=relv_sb[:, p, 0:1], op=mybir.AluOpType.subtract)

            # ---- per-pass row tiles: 3 k-variants (k+rel[a]), vf, qt, E ----
            kv = [[rows_pool.tile([128, DP, HBP, WP], BF16, tag=f"kv{p}{a}",
                                  name=f"kv{p}{a}") for a in range(K)]
                  for p in range(NPASS)]
            vf = [rows_pool.tile([128, DP, HBP, WP], BF16, tag=f"vf{p}", name=f"vf{p}")
                  for p in range(NPASS)]
            qt = [rows_pool.tile([128, DLOC, HB, WP], BF16, tag=f"qt{p}", name=f"qt{p}")
                  for p in range(NPASS)]
            et = [rows_pool.tile([128, DLOC, HB, WP], F32, tag=f"et{p}", name=f"et{p}")
                  for p in range(NPASS)]
            for p in range(NPASS):
                nc.scalar.memzero(qt[p])  # pad cols stay 0 -> finite exp

            # proj psum ring: proj pool (2 bufs) + all 6 accs tags -> 8 slots
            psidx = [0]

            def proj_tile(n, wide=False):
                if not wide:
                    return proj_ps.tile([128, n], F32, tag="ps", name="ps")
                k = psidx[0] % 8
                psidx[0] += 1
                if k < 2:
                    return proj_ps.tile([128, n], F32, tag="ps", name="ps")
                tag = f"den{k - 2}" if k < 5 else f"num{k - 5}"
                return accs.tile([128, n], F32, tag=tag, name="ps_" + tag)

            def _e_tile(p):
                # E = exp(q) f32, whole tile (pads are exp(0)=1, harmless)
                nc.scalar.activation(out=et[p], in_=qt[p],
                                     func=mybir.ActivationFunctionType.Exp)

            def _k_chunks(p, lo, hi, wide):
                var_eng = nc.vector if p == 0 else nc.gpsimd
                var_act = p in VAR_ACT
                v0f = kv[p][0].rearrange("r d h w -> r (d h w)")
                vaf = [kv[p][a].rearrange("r d h w -> r (d h w)") for a in range(K)]
                for i in range(lo, hi):
                    ps = proj_tile(440, wide)
                    sl = slice(i * 440, (i + 1) * 440)
                    for u in range(2):
                        xflat = x4_sb[:, u].rearrange("c d h w -> c (d h w)")
                        nc.tensor.matmul(ps[64 * u:64 * u + 64], wp_sb[:, p],
                                         xflat[:, sl], start=True, stop=True)
                    nc.scalar.activation(out=v0f[:, sl], in_=ps,
                                         func=mybir.ActivationFunctionType.Identity,
                                         bias=relv_sb[:, p, 0:1])
                    for a in (1, 2):
                        if var_act:
                            nc.scalar.activation(
                                out=vaf[a][:, sl], in_=v0f[:, sl],
                                func=mybir.ActivationFunctionType.Identity,
                                bias=reld_sb[:, p, a - 1:a])
                        else:
                            var_eng.tensor_scalar(
                                out=vaf[a][:, sl], in0=v0f[:, sl],
                                scalar1=reld_sb[:, p, a - 1:a], scalar2=None,
                                op0=mybir.AluOpType.add)

            def _v_chunks(p, lo, hi, wide):
                dflat = vf[p].rearrange("r d h w -> r (d h w)")
                for i in range(lo, hi):
                    ps = proj_tile(440, wide)
                    sl = slice(i * 440, (i + 1) * 440)
                    for u in range(2):
                        xflat = x4_sb[:, u].rearrange("c d h w -> c (d h w)")
                        nc.tensor.matmul(ps[64 * u:64 * u + 64], wp_sb[:, 3 + p],
                                         xflat[:, sl], start=True, stop=True)
                    nc.scalar.copy(dflat[:, sl], ps)

            def _q_chunks(p, wide):
                for i in range(3):
                    ps = proj_tile(440, wide)
                    for u in range(2):
                        rhs = x4_sb[:, u, 1:1 + DLOC, 1 + 2 * i:3 + 2 * i, 1:1 + W]
                        nc.tensor.matmul(ps[64 * u:64 * u + 64, 0:384],
                                         wp_sb[:, 6 + p], rhs,
                                         start=True, stop=True)
                    nc.scalar.copy(qt[p][:, :, 2 * i:2 * i + 2, 0:W],
                                   ps[:, 0:384])

            def stage_pieces(p, wide=False):
                yield lambda: (_k_chunks(p, 0, 3, wide), _q_chunks(p, wide),
                               _e_tile(p))
                yield lambda: _k_chunks(p, 3, 6, wide)
                yield lambda: _v_chunks(p, 0, 6, wide)

            def _prologue():
                p = 0
                v0f = kv[p][0].rearrange("r d h w -> r (d h w)")
                vaf = [kv[p][a].rearrange("r d h w -> r (d h w)")
                       for a in range(K)]

                def k_mm(i):
                    ps = proj_tile(440, True)
                    sl = slice(i * 440, (i + 1) * 440)
                    for u in range(2):
                        xflat = x4_sb[:, u].rearrange("c d h w -> c (d h w)")
                        nc.tensor.matmul(ps[64 * u:64 * u + 64], wp_sb[:, p],
                                         xflat[:, sl], start=True, stop=True)
                    return ps, sl

                def k_evac(ps, sl, eng):
                    if eng == "act":
                        nc.scalar.activation(
                            out=v0f[:, sl], in_=ps,
                            func=mybir.ActivationFunctionType.Identity,
                            bias=relv_sb[:, p, 0:1])
                    else:
                        nc.vector.tensor_scalar(
                            out=v0f[:, sl], in0=ps,
                            scalar1=relv_sb[:, p, 0:1], scalar2=None,
                            op0=mybir.AluOpType.add)
                    for a in (1, 2):
                        nc.vector.tensor_scalar(
                            out=vaf[a][:, sl], in0=v0f[:, sl],
                            scalar1=reld_sb[:, p, a - 1:a], scalar2=None,
                            op0=mybir.AluOpType.add)

                # k chunks 0,1 only need the first x4 DMA chunk
                kps = {}
                for i in (0, 1):
                    kps[i] = k_mm(i)
                _q_chunks(p, True)
                for i in (2, 3, 4, 5):
                    kps[i] = k_mm(i)
                k_evac(*kps[0], "act")
                k_evac(*kps[1], "dve")
                k_evac(*kps[2], "act")
                _e_tile(p)
                k_evac(*kps[3], "dve")
                k_evac(*kps[4], "act")
                k_evac(*kps[5], "dve")
                # v projections + DVE evacs (DVE is idle pre-attention)
                dflat = vf[p].rearrange("r d h w -> r (d h w)")
                for i in range(6):
                    ps = proj_tile(440, True)
                    sl = slice(i * 440, (i + 1) * 440)
                    for u in range(2):
                        xflat = x4_sb[:, u].rearrange("c d h w -> c (d h w)")
                        nc.tensor.matmul(ps[64 * u:64 * u + 64], wp_sb[:, 3 + p],
                                         xflat[:, sl], start=True, stop=True)
                    if i % 2 == 0:
                        nc.vector.tensor_copy(out=dflat[:, sl], in_=ps)
                    else:
                        nc.scalar.copy(dflat[:, sl], ps)

            _prologue()

            # ---- attention (pass p+1's staging pieces are emitted between
            # pass p's groups so no engine queue blocks behind staging) ----
            pending = []
            prod_pre = {}      # (p, g) -> pre-allocated scr tile
            produced = set()   # (p, s) produced ahead into prod_pre tile

            def geom(p, g, hj2):
                # Tap geometry per pass: group g and subgroup hj2 fix two
                # axes; the third rides one merged 3-plane op. The rel
                # variant is constant per (g, hj2) in every pass:
                #   p0: taps (dj=t, hj=hj2, wj=g), variant g, tap stride 528
                #   p1: taps (dj=g, hj=hj2, wj=t), variant g, tap stride 1
                #   p2: taps (dj=g, hj=hj2, wj=t), variant hj2, tap stride 1
                if p == 0:
                    return g, HBP * WP, 0, hj2 * WP + g
                return (g if p == 1 else hj2), 1, g, hj2 * WP

            def produce(p, g, hj2, scr):
                s = 3 * g + hj2
                a, ts, dbase, fbase = geom(p, g, hj2)
                kvf = kv[p][a].rearrange("r d h w -> r d (h w)")
                kb = kvf[:, dbase, fbase:fbase + 1]
                kwin = bass.AP(
                    tensor=kb.tensor, offset=kb.offset,
                    ap=[kb.ap[0], [ts, K], [HBP * WP, DLOC], [1, FL]])
                sub = scr[:, 3 * hj2:3 * hj2 + 3]
                if s in PATHP[p]:
                    etf = et[p].rearrange("r d h w -> r d (h w)")
                    eb = etf[:, 0:1, 0:1]
                    ewin = bass.AP(
                        tensor=eb.tensor, offset=eb.offset,
                        ap=[eb.ap[0], [0, K], [HB * WP, DLOC], [1, FL]])
                    nc.gpsimd.tensor_tensor(out=sub, in0=ewin, in1=kwin,
                                            op=mybir.AluOpType.pow)
                else:
                    qtf = qt[p].rearrange("r d h w -> r d (h w)")
                    qb = qtf[:, 0:1, 0:1]
                    qwin = bass.AP(
                        tensor=qb.tensor, offset=qb.offset,
                        ap=[qb.ap[0], [0, K], [HB * WP, DLOC], [1, FL]])
                    nc.vector.tensor_tensor(out=sub, in0=kwin, in1=qwin,
                                            op=mybir.AluOpType.mult)
                    nc.scalar.activation(out=sub, in_=sub,
                                         func=mybir.ActivationFunctionType.Exp)

            for p in range(NPASS):
                vff = vf[p].rearrange("r d h w -> r d (h w)")
                den = [accs.tile([128, 512], F32, tag=f"den{c}", name=f"den{c}")
                       for c in range(DLOC)]
                num = [accs.tile([128, 512], F32, tag=f"num{c}", name=f"num{c}")
                       for c in range(DLOC)]
                nxt = list(stage_pieces(p + 1)) if p + 1 < NPASS else []
                rcp = None if USE_DIVIDE else attn.tile(
                    [128, DLOC, FL], F32, tag="rcp", bufs=2, name="rcp")
                num_pending = []
                # start/stop bookkeeping for the PE accumulation chains
                nden = 27 - len(MERGE_SCR)
                nnum = 27 - len(MERGE_EV[p])
                den_emitted = [0]
                num_emitted = [0]

                def emit_den(scr, hj2, planes):
                    for jj2 in planes:
                        jj = 3 * hj2 + jj2
                        first = den_emitted[0] == 0
                        den_emitted[0] += 1
                        last = den_emitted[0] == nden
                        for c in range(DLOC):
                            rb = scr[:, jj, c * FL:c * FL + 1]
                            rt = bass.AP(tensor=rb.tensor, offset=rb.offset,
                                         ap=[rb.ap[0], [WP, HB], [1, W]])
                            nc.tensor.matmul(
                                den[c][:, 0:HB * W], id_sb, rt,
                                start=first, stop=last)

                for g in range(K):
                    scr = prod_pre.pop((p, g), None)
                    if scr is None:
                        scr = attn.tile([128, 9, NV], BF16, tag="scr",
                                        bufs=3, name="scr")
                    for hj2 in HJ2_ORDER:
                        s = 3 * g + hj2
                        if (p, s) not in produced:
                            produce(p, g, hj2, scr)
                        if s == 0 and pending:
                            pending.pop(0)()
                        a, ts, dbase, fbase = geom(p, g, hj2)
                        sub = scr[:, 3 * hj2:3 * hj2 + 3]
                        if s not in MERGE_SCR:
                            emit_den(scr, hj2, (0, 1, 2))
                        ev = attn.tile([128, K, NV], BF16, tag="ev",
                                       bufs=EV_BUFS, name="ev")
                        base = vff[:, dbase, fbase:fbase + 1]
                        vwin = bass.AP(
                            tensor=base.tensor, offset=base.offset,
                            ap=[base.ap[0], [ts, K], [HBP * WP, DLOC], [1, FL]])
                        eng = nc.gpsimd if s in EV_POOL[p] else nc.vector
                        eng.tensor_tensor(
                            out=ev, in0=sub, in1=vwin,
                            op=mybir.AluOpType.mult)
                        if s in MERGE_SCR:
                            # den planes 0+1 merged on DVE (after e*v read)
                            nc.vector.tensor_tensor(
                                out=sub[:, 0], in0=sub[:, 0], in1=sub[:, 1],
                                op=mybir.AluOpType.add)
                            emit_den(scr, hj2, (0, 2))
                        merged = s in MERGE_EV[p]
                        if merged:
                            # ev[0] += ev[1]: PE num-chain then only
                            # needs planes {0, 2} of this subgroup
                            meng = nc.gpsimd if s in MERGE_POOL else nc.vector
                            meng.tensor_tensor(
                                out=ev[:, 0], in0=ev[:, 0], in1=ev[:, 1],
                                op=mybir.AluOpType.add)
                        def emit_num(ev=ev, g=g, hj2=hj2, merged=merged):
                            planes = (0, 2) if merged else (0, 1, 2)
                            def ev_rt(jj2, c):
                                eb = ev[:, jj2, c * FL:c * FL + 1]
                                return bass.AP(
                                    tensor=eb.tensor, offset=eb.offset,
                                    ap=[eb.ap[0], [WP, HB], [1, W]])
                            if g == 2 and hj2 == 2:
                                # c-major close: chunk 0's chain stops first
                                # so its normalize+DMA overlaps remaining mms
                                for c in range(DLOC):
                                    for jj2 in planes:
                                        num_emitted[0] += 1
                                        nc.tensor.matmul(
                                            num[c][:, 0:HB * W], id_sb,
                                            ev_rt(jj2, c),
                                            start=False, stop=(jj2 == 2))
                            else:
                                for jj2 in planes:
                                    first = num_emitted[0] == 0
                                    num_emitted[0] += 1
                                    for c in range(DLOC):
                                        nc.tensor.matmul(
                                            num[c][:, 0:HB * W], id_sb,
                                            ev_rt(jj2, c),
                                            start=first, stop=False)
                        # defer num-mms one subgroup so a late e*v never
                        # head-of-line-blocks the next subgroup's den-mms
                        # on the in-order PE queue
                        num_pending.append(emit_num)
                        if len(num_pending) > NUM_DEFER:
                            num_pending.pop(0)()
                    if g == 2:
                        while num_pending:
                            num_pending.pop(0)()
                    if g < len(nxt):
                        nxt[g]()
                    if g == 2 and not USE_DIVIDE:
                        # den chains are closed; reciprocals can overlap the
                        # remaining num accumulation
                        for c in range(DLOC):
                            nc.vector.reciprocal_approx_fast(
                                out=rcp[:, c, 0:HB * W], in_=den[c][:, 0:HB * W])
                    if g == 2 and p + 1 < NPASS:
                        # lookahead: produce next pass's first group while
                        # this pass drains, hiding the boundary production
                        nscr = attn.tile([128, 9, NV], BF16, tag="scr",
                                         bufs=3, name="scr")
                        prod_pre[(p + 1, 0)] = nscr
                        for hj2n in range(K):
                            produce(p + 1, 0, hj2n, nscr)
                            produced.add((p + 1, hj2n))

                def finals(p=p, num=num, den=den, rcp=rcp):
                    out_t = outs.tile([128, DLOC, HB, W], F32, tag="out",
                                      bufs=2, name="out_t")
                    ns = outs.tile([128, DLOC, HB, W], F32, tag="ns",
                                   bufs=2, name="ns") if FMUL_OFF else None
                    for c in range(DLOC):
                        if c in FMUL_OFF:
                            nc.scalar.copy(ns[:, c], num[c][:, 0:HB * W])
                            nc.gpsimd.tensor_tensor(
                                out=out_t[:, c], in0=ns[:, c],
                                in1=rcp[:, c, 0:HB * W],
                                op=mybir.AluOpType.mult)
                        else:
                            nc.vector.tensor_tensor(
                                out=out_t[:, c], in0=num[c][:, 0:HB * W],
                                in1=rcp[:, c, 0:HB * W],
                                op=mybir.AluOpType.mult)
                        nc.sync.dma_start(out=y[p, :, c], in_=out_t[:, c])
                pending.append(finals)
            while pending:
                pending.pop(0)()
    nc.compile()
    return nc


def _host_prep(x, Wq, Wk, Wv, rel_h, rel_w, rel_d):
    import ml_dtypes
    tobf = lambda a: np.ascontiguousarray(a).astype(ml_dtypes.bfloat16)

    x = np.asarray(x, np.float32).reshape(CIN, D, H, W)
    xp = np.pad(x, ((0, 0), (1, 1), (1, 1), (1, 1)))  # (32, 26, 50, 66)

    # block-diagonal lhsT: rows 32g+cin -> cols 16g+i, chans 16p+i of W_kvq
    wproj = np.zeros((128, 9, 64), np.float32)
    for kvq, Wm in enumerate((Wk, Wv, Wq)):
        WT = np.asarray(Wm, np.float32).T  # (CIN, COUT)
        for p in range(NPASS):
            for g in range(4):
                wproj[32 * g:32 * g + 32, 3 * kvq + p,
                      16 * g:16 * g + 16] = WT[:, 16 * p:16 * p + 16]

    # relv[p, r, a]: row r (chan 16p + r%16) rel value at axis-index a.
    # chan type 0 (0:16) uses rel_d (varies over wj), type 1 rel_h (dj),
    # type 2 rel_w (hj).
    rel_d2 = np.asarray(rel_d, np.float32).reshape(C3, K)
    rel_h2 = np.asarray(rel_h, np.float32).reshape(C3, K)
    rel_w2 = np.asarray(rel_w, np.float32).reshape(C3, K)
    rel_by_type = (rel_d2, rel_h2, rel_w2)
    relv = np.zeros((NPASS, 128, K), np.float32)
    for p in range(NPASS):
        for r in range(128):
            relv[p, r] = rel_by_type[p][r % 16]

    wproj_bf = tobf(wproj)
    ident_bf = tobf(np.eye(128, dtype=np.float32))

    in_maps = []
    for i in range(NCORES):
        slab = xp[:, 3 * i:3 * i + DP]  # (32, 5, 50, 66)
        xb = np.empty((CIN, NS, DP, HBP, WP), np.float32)
        for s in range(NS):
            xb[:, s] = slab[:, :, HB * s:HB * s + HBP, :]
        x4 = np.empty((128, 2, DP, HBP, WP), np.float32)
        for u in range(2):
            for g in range(4):
                x4[32 * g:32 * g + 32, u] = xb[:, 4 * u + g]
        in_maps.append({
            "x4": tobf(x4), "wproj": wproj_bf, "relv": relv,
            "ident": ident_bf,
        })
    return in_maps


def kernel(x, Wq, Wk, Wv, rel_h, rel_w, rel_d, trace=False):
    in_maps = _host_prep(x, Wq, Wk, Wv, rel_h, rel_w, rel_d)
    if "nc" not in _CACHE:
        _CACHE["nc"] = build_program()
    res = run_bass_kernel_spmd(
        _CACHE["nc"], in_maps, core_ids=list(range(NCORES)), trace=trace)
    out = np.zeros((COUT, D, H, W), np.float32)
    for i in range(NCORES):
        yv = np.asarray(res.results[i]["y"])  # (NPASS, 128, DLOC, HB, W)
        # pass p, row r -> block r//16, chan 16p + r%16
        v = yv.reshape(NPASS, NS, 16, DLOC, HB, W)       # [p, b, i, d, h, w]
        v = v.transpose(0, 2, 3, 1, 4, 5)                 # [p, i, d, b, h, w]
        out[:, 3 * i:3 * i + DLOC] = v.reshape(COUT, DLOC, H, W)
    if trace:
        _CACHE["last"] = res
    return out.reshape(1, COUT, D, H, W)
